# revision 4
# baseline (speedup 1.0000x reference)
"""nn_BaseModel mLSTM kernel for 8 TRN2 NeuronCores.

Strategy (data-parallel per sharding hint): batch is sharded 8 ways across
cores for the recurrent mLSTM (sequential in time); weights are replicated.
The recurrence runs via jax (XLA on the neuron cores) sharded over devices;
the classifier tail runs as a Bass SPMD kernel on cores 0-7 via
bass_utils.run_bass_kernel_spmd.
"""
import numpy as np

PAD = 26
H = 1900
B = 256
T_EPI = 25
T_TOT = 153
EMB = 10
IC = 2 * H
N_CORES = 8

_BASS_CACHE = {}


def _build_classifier_kernel():
    """Bass SPMD kernel: per-core batch shard of the classifier.

    In:  x  [32, 3800]  (concat[tot_h, epi_h] after lrelu+bn1 folding on host?
         no — kernel does lrelu/bn/matmuls on device)
         packed params.
    Out: y  [1, 32]
    """
    import concourse.bacc as bacc
    import concourse.mybir as mybir
    from concourse.tile import TileContext

    BS = B // N_CORES  # 32 rows per core
    nc = bacc.Bacc("TRN2", target_bir_lowering=False, num_devices=N_CORES)
    # transposed feature-major layout: xT [3840 pad, 32] -> 30 k-tiles of 128
    KT = 30  # 3840 / 128
    xT = nc.declare_dram_parameter("xT", [KT * 128, BS], mybir.dt.float32, isOutput=False)
    # bn1 affine folded: s1,o1 [3840]; W1 [3840, 384]; bn2 affine s2,o2 [384];
    # W2 [384]; b2 scalar. Packed host-side:
    w1 = nc.declare_dram_parameter("w1", [KT * 128, 384], mybir.dt.float32, isOutput=False)
    s1 = nc.declare_dram_parameter("s1", [KT * 128, 1], mybir.dt.float32, isOutput=False)
    o1 = nc.declare_dram_parameter("o1", [KT * 128, 1], mybir.dt.float32, isOutput=False)
    s2 = nc.declare_dram_parameter("s2", [3 * 128, 1], mybir.dt.float32, isOutput=False)
    o2 = nc.declare_dram_parameter("o2", [3 * 128, 1], mybir.dt.float32, isOutput=False)
    w2 = nc.declare_dram_parameter("w2", [3 * 128, 1], mybir.dt.float32, isOutput=False)
    b2 = nc.declare_dram_parameter("b2", [1, 1], mybir.dt.float32, isOutput=False)
    y = nc.declare_dram_parameter("y", [1, BS], mybir.dt.float32, isOutput=True)

    with TileContext(nc) as tc:
        with (
            tc.tile_pool(name="sb", bufs=1) as sb,
            tc.tile_pool(name="ps", bufs=1, space="PSUM") as ps,
        ):
            xt = sb.tile([128, KT * BS], mybir.dt.float32)
            nc.sync.dma_start(out=xt[:, :].rearrange("p (k b) -> p k b", k=KT),
                              in_=xT.ap().rearrange("(k p) b -> p k b", p=128))
            w1t = sb.tile([128, KT * 384], mybir.dt.float32)
            nc.sync.dma_start(out=w1t[:, :].rearrange("p (k m) -> p k m", k=KT),
                              in_=w1.ap().rearrange("(k p) m -> p k m", p=128))
            s1t = sb.tile([128, KT], mybir.dt.float32)
            nc.sync.dma_start(out=s1t[:, :].rearrange("p (k o) -> p k o", o=1),
                              in_=s1.ap().rearrange("(k p) o -> p k o", p=128))
            o1t = sb.tile([128, KT], mybir.dt.float32)
            nc.sync.dma_start(out=o1t[:, :].rearrange("p (k o) -> p k o", o=1),
                              in_=o1.ap().rearrange("(k p) o -> p k o", p=128))
            s2t = sb.tile([128, 3], mybir.dt.float32)
            nc.sync.dma_start(out=s2t[:, :].rearrange("p (k o) -> p k o", o=1),
                              in_=s2.ap().rearrange("(k p) o -> p k o", p=128))
            o2t = sb.tile([128, 3], mybir.dt.float32)
            nc.sync.dma_start(out=o2t[:, :].rearrange("p (k o) -> p k o", o=1),
                              in_=o2.ap().rearrange("(k p) o -> p k o", p=128))
            w2t = sb.tile([128, 3], mybir.dt.float32)
            nc.sync.dma_start(out=w2t[:, :].rearrange("p (k o) -> p k o", o=1),
                              in_=w2.ap().rearrange("(k p) o -> p k o", p=128))
            b2t = sb.tile([1, 1], mybir.dt.float32)
            nc.sync.dma_start(out=b2t[:, :], in_=b2[:, :])

            # stage 1: u = lrelu(x) * s1 + o1 per feature row
            ut = sb.tile([128, KT * BS], mybir.dt.float32)
            for k in range(KT):
                xs = xt[:, k * BS:(k + 1) * BS]
                us = ut[:, k * BS:(k + 1) * BS]
                nc.vector.tensor_scalar_mul(us, xs, 0.3)
                nc.vector.tensor_tensor(us, xs, us, mybir.AluOpType.max)
                nc.vector.tensor_scalar(us, us, s1t[:, k:k + 1], o1t[:, k:k + 1],
                                        mybir.AluOpType.mult, mybir.AluOpType.add)
            # z1 = u.T @ W1 -> transposed: z1T [384, 32] = sum_k W1[k].T @ u[k]
            z1ps = []
            for m in range(3):
                z1p_m = ps.tile([128, BS], mybir.dt.float32, tag=f"z1p{m}")
                z1ps.append(z1p_m)
                for k in range(KT):
                    nc.tensor.matmul(
                        z1p_m[:, :],
                        w1t[:, k * 384 + m * 128: k * 384 + (m + 1) * 128],
                        ut[:, k * BS:(k + 1) * BS],
                        start=(k == 0), stop=(k == KT - 1))
            # stage 2: v = lrelu(z1) * s2 + o2 ; y = v.T @ W2 + b2
            vt = sb.tile([128, 3 * BS], mybir.dt.float32)
            for m in range(3):
                zs = z1ps[m][:, :]
                vs = vt[:, m * BS:(m + 1) * BS]
                nc.vector.tensor_scalar_mul(vs, zs, 0.3)
                nc.vector.tensor_tensor(vs, zs, vs, mybir.AluOpType.max)
                nc.vector.tensor_scalar(vs, vs, s2t[:, m:m + 1], o2t[:, m:m + 1],
                                        mybir.AluOpType.mult, mybir.AluOpType.add)
            yp = ps.tile([1, BS], mybir.dt.float32)
            for m in range(3):
                nc.tensor.matmul(yp[:, :], w2t[:, m:m + 1], vt[:, m * BS:(m + 1) * BS],
                                 start=(m == 0), stop=(m == 2))
            yt = sb.tile([1, BS], mybir.dt.float32)
            nc.vector.tensor_scalar_add(yt[:, :], yp[:, :], b2t[:1, :1])
            nc.sync.dma_start(out=y[:, :], in_=yt[:, :])
    nc.compile()
    return nc


def _wn(w, g):
    n = np.sqrt(np.maximum((w.astype(np.float64) ** 2).sum(axis=0, keepdims=True), 1e-12))
    return (w * (g / n)).astype(np.float32)


def kernel(epitope_x, left_antigen_x, right_antigen_x, total_antigen_x, embed,
           wx, wh, wmx, wmh, b, gx, gh, gmx, gmh,
           bn1_gamma, bn1_beta, bn1_mean, bn1_var, W1, b1,
           bn2_gamma, bn2_beta, bn2_mean, bn2_var, W2, b2):
    import jax
    import jax.numpy as jnp
    from concourse import bass_utils

    epitope_x = np.asarray(epitope_x)
    left_antigen_x = np.asarray(left_antigen_x)
    right_antigen_x = np.asarray(right_antigen_x)
    total_antigen_x = np.asarray(total_antigen_x)
    embed = np.asarray(embed, np.float32)
    wxn = _wn(np.asarray(wx, np.float32), np.asarray(gx, np.float32))
    whn = _wn(np.asarray(wh, np.float32), np.asarray(gh, np.float32))
    wmxn = _wn(np.asarray(wmx, np.float32), np.asarray(gmx, np.float32))
    wmhn = _wn(np.asarray(wmh, np.float32), np.asarray(gmh, np.float32))
    bv = np.asarray(b, np.float32)

    epi_len = (epitope_x != PAD).sum(axis=1).astype(np.int64)
    left_len = np.maximum((left_antigen_x != PAD).sum(axis=1), 1).astype(np.int64)
    right_len = np.maximum((right_antigen_x != PAD).sum(axis=1), 1).astype(np.int64)
    tot_len = epi_len + left_len + right_len
    ei = np.clip(epi_len - 1, 0, T_EPI - 1)
    ti = np.clip(tot_len - 1, 0, T_TOT - 1)

    epi_emb = embed[epitope_x]      # [B, 25, 10]
    tot_emb = embed[total_antigen_x]  # [B, 153, 10]

    # ---- recurrence: data-parallel across the 8 cores via jax shard_map ----
    devs = jax.devices()[:N_CORES]
    mesh = jax.sharding.Mesh(np.array(devs), ("d",))

    def mlstm_last(xs, idx, wxn_, whn_, wmxn_, wmhn_, b_):
        # xs [bs, T, E]; returns h at per-row idx
        xsT = jnp.swapaxes(xs, 0, 1)

        def step(carry, x):
            c, h = carry
            m = (x @ wmxn_) * (h @ wmhn_)
            z = x @ wxn_ + m @ whn_ + b_
            i, f, o, u = jnp.split(z, 4, axis=1)
            c = jax.nn.sigmoid(f) * c + jax.nn.sigmoid(i) * jnp.tanh(u)
            h = jax.nn.sigmoid(o) * jnp.tanh(c)
            return (c, h), h

        bs = xs.shape[0]
        init = (jnp.zeros((bs, H), xs.dtype), jnp.zeros((bs, H), xs.dtype))
        _, hs = jax.lax.scan(step, init, xsT)
        hs = jnp.swapaxes(hs, 0, 1)
        return jnp.take_along_axis(hs, idx[:, None, None], axis=1)[:, 0]

    from jax.experimental.shard_map import shard_map
    from jax.sharding import PartitionSpec as P

    @jax.jit
    def run(epi_e, tot_e, ei_, ti_, wxn_, whn_, wmxn_, wmhn_, b_):
        f = shard_map(
            lambda ee, te, e_i, t_i: (
                mlstm_last(ee, e_i, wxn_, whn_, wmxn_, wmhn_, b_),
                mlstm_last(te, t_i, wxn_, whn_, wmxn_, wmhn_, b_),
            ),
            mesh=mesh,
            in_specs=(P("d"), P("d"), P("d"), P("d")),
            out_specs=(P("d"), P("d")),
            check_rep=False,
        )
        return f(epi_e, tot_e, ei_, ti_)

    epi_h, tot_h = run(jnp.asarray(epi_emb), jnp.asarray(tot_emb),
                       jnp.asarray(ei), jnp.asarray(ti),
                       jnp.asarray(wxn), jnp.asarray(whn), jnp.asarray(wmxn),
                       jnp.asarray(wmhn), jnp.asarray(bv))
    epi_h = np.asarray(epi_h)
    tot_h = np.asarray(tot_h)

    x = np.concatenate([tot_h, epi_h], axis=-1)  # [B, 3800]

    # ---- classifier on device via Bass SPMD ----
    if "clf" not in _BASS_CACHE:
        _BASS_CACHE["clf"] = _build_classifier_kernel()
    nc = _BASS_CACHE["clf"]

    ICP = 30 * 128  # 3840
    s1v = (np.asarray(bn1_gamma, np.float32) /
           np.sqrt(np.asarray(bn1_var, np.float32) + 1e-3))
    o1v = np.asarray(bn1_beta, np.float32) - np.asarray(bn1_mean, np.float32) * s1v
    s2v = (np.asarray(bn2_gamma, np.float32) /
           np.sqrt(np.asarray(bn2_var, np.float32) + 1e-3))
    o2v = np.asarray(bn2_beta, np.float32) - np.asarray(bn2_mean, np.float32) * s2v
    # fold b1 into stage-2 input: v = lrelu(z1 + b1)*s2+o2 -> shift o? b1 enters
    # before lrelu, so fold b1 into W1 via an extra input row? Simplest: add a
    # constant feature: append row with value 1 to xT and b1 row to W1.
    s1p = np.zeros((ICP, 1), np.float32)
    o1p = np.zeros((ICP, 1), np.float32)
    s1p[:IC, 0] = s1v
    o1p[:IC, 0] = o1v
    # constant-one feature at row IC: s1=0, o1=1 -> u=1; W1 row = b1
    s1p[IC, 0] = 0.0
    o1p[IC, 0] = 1.0
    w1p = np.zeros((ICP, 384), np.float32)
    w1p[:IC, :380] = np.asarray(W1, np.float32)
    w1p[IC, :380] = np.asarray(b1, np.float32)
    s2p = np.zeros((384, 1), np.float32)
    o2p = np.zeros((384, 1), np.float32)
    s2p[:380, 0] = s2v
    o2p[:380, 0] = o2v
    w2p = np.zeros((384, 1), np.float32)
    w2p[:380, 0] = np.asarray(W2, np.float32)[:, 0]
    b2p = np.asarray(b2, np.float32).reshape(1, 1)

    BS = B // N_CORES
    in_maps = []
    for c in range(N_CORES):
        xs = np.zeros((ICP, BS), np.float32)
        xs[:IC, :] = x[c * BS:(c + 1) * BS, :].T
        xs[IC, :] = 1.0
        in_maps.append({
            "xT": xs, "w1": w1p, "s1": s1p, "o1": o1p,
            "s2": s2p, "o2": o2p, "w2": w2p, "b2": b2p,
        })
    res = bass_utils.run_bass_kernel_spmd(nc, in_maps, core_ids=list(range(N_CORES)))
    y = np.concatenate([res.results[c]["y"][0] for c in range(N_CORES)])
    return y.astype(np.float32)


# revision 5
# speedup vs baseline: 1.0316x; 1.0316x over previous
"""nn_BaseModel mLSTM kernel for 8 TRN2 NeuronCores.

Strategy (data-parallel per sharding hint): batch is sharded 8 ways across
cores for the recurrent mLSTM (sequential in time); weights are replicated.
The recurrence runs via jax (XLA on the neuron cores) sharded over devices;
the classifier tail runs as a Bass SPMD kernel on cores 0-7 via
bass_utils.run_bass_kernel_spmd.
"""
import numpy as np

PAD = 26
H = 1900
B = 256
T_EPI = 25
T_TOT = 153
EMB = 10
IC = 2 * H
N_CORES = 8

_BASS_CACHE = {}


def _build_classifier_kernel():
    """Bass SPMD kernel: per-core batch shard of the classifier.

    In:  x  [32, 3800]  (concat[tot_h, epi_h] after lrelu+bn1 folding on host?
         no — kernel does lrelu/bn/matmuls on device)
         packed params.
    Out: y  [1, 32]
    """
    import concourse.bacc as bacc
    import concourse.mybir as mybir
    from concourse.tile import TileContext

    BS = B // N_CORES  # 32 rows per core
    nc = bacc.Bacc("TRN2", target_bir_lowering=False, num_devices=N_CORES)
    # transposed feature-major layout: xT [3840 pad, 32] -> 30 k-tiles of 128
    KT = 30  # 3840 / 128
    xT = nc.declare_dram_parameter("xT", [KT * 128, BS], mybir.dt.float32, isOutput=False)
    # bn1 affine folded: s1,o1 [3840]; W1 [3840, 384]; bn2 affine s2,o2 [384];
    # W2 [384]; b2 scalar. Packed host-side:
    w1 = nc.declare_dram_parameter("w1", [KT * 128, 384], mybir.dt.float32, isOutput=False)
    s1 = nc.declare_dram_parameter("s1", [KT * 128, 1], mybir.dt.float32, isOutput=False)
    o1 = nc.declare_dram_parameter("o1", [KT * 128, 1], mybir.dt.float32, isOutput=False)
    s2 = nc.declare_dram_parameter("s2", [3 * 128, 1], mybir.dt.float32, isOutput=False)
    o2 = nc.declare_dram_parameter("o2", [3 * 128, 1], mybir.dt.float32, isOutput=False)
    w2 = nc.declare_dram_parameter("w2", [3 * 128, 1], mybir.dt.float32, isOutput=False)
    b2 = nc.declare_dram_parameter("b2", [1, 1], mybir.dt.float32, isOutput=False)
    y = nc.declare_dram_parameter("y", [1, BS], mybir.dt.float32, isOutput=True)

    with TileContext(nc) as tc:
        with (
            tc.tile_pool(name="sb", bufs=1) as sb,
            tc.tile_pool(name="ps", bufs=1, space="PSUM") as ps,
        ):
            xt = sb.tile([128, KT * BS], mybir.dt.float32)
            nc.sync.dma_start(out=xt[:, :].rearrange("p (k b) -> p k b", k=KT),
                              in_=xT.ap().rearrange("(k p) b -> p k b", p=128))
            w1t = sb.tile([128, KT * 384], mybir.dt.float32)
            nc.sync.dma_start(out=w1t[:, :].rearrange("p (k m) -> p k m", k=KT),
                              in_=w1.ap().rearrange("(k p) m -> p k m", p=128))
            s1t = sb.tile([128, KT], mybir.dt.float32)
            nc.sync.dma_start(out=s1t[:, :].rearrange("p (k o) -> p k o", o=1),
                              in_=s1.ap().rearrange("(k p) o -> p k o", p=128))
            o1t = sb.tile([128, KT], mybir.dt.float32)
            nc.sync.dma_start(out=o1t[:, :].rearrange("p (k o) -> p k o", o=1),
                              in_=o1.ap().rearrange("(k p) o -> p k o", p=128))
            s2t = sb.tile([128, 3], mybir.dt.float32)
            nc.sync.dma_start(out=s2t[:, :].rearrange("p (k o) -> p k o", o=1),
                              in_=s2.ap().rearrange("(k p) o -> p k o", p=128))
            o2t = sb.tile([128, 3], mybir.dt.float32)
            nc.sync.dma_start(out=o2t[:, :].rearrange("p (k o) -> p k o", o=1),
                              in_=o2.ap().rearrange("(k p) o -> p k o", p=128))
            w2t = sb.tile([128, 3], mybir.dt.float32)
            nc.sync.dma_start(out=w2t[:, :].rearrange("p (k o) -> p k o", o=1),
                              in_=w2.ap().rearrange("(k p) o -> p k o", p=128))
            b2t = sb.tile([1, 1], mybir.dt.float32)
            nc.sync.dma_start(out=b2t[:, :], in_=b2[:, :])

            # stage 1: u = lrelu(x) * s1 + o1 per feature row
            ut = sb.tile([128, KT * BS], mybir.dt.float32)
            for k in range(KT):
                xs = xt[:, k * BS:(k + 1) * BS]
                us = ut[:, k * BS:(k + 1) * BS]
                nc.vector.tensor_scalar_mul(us, xs, 0.3)
                nc.vector.tensor_tensor(us, xs, us, mybir.AluOpType.max)
                nc.vector.tensor_scalar(us, us, s1t[:, k:k + 1], o1t[:, k:k + 1],
                                        mybir.AluOpType.mult, mybir.AluOpType.add)
            # z1 = u.T @ W1 -> transposed: z1T [384, 32] = sum_k W1[k].T @ u[k]
            z1ps = []
            for m in range(3):
                z1p_m = ps.tile([128, BS], mybir.dt.float32, tag=f"z1p{m}")
                z1ps.append(z1p_m)
                for k in range(KT):
                    nc.tensor.matmul(
                        z1p_m[:, :],
                        w1t[:, k * 384 + m * 128: k * 384 + (m + 1) * 128],
                        ut[:, k * BS:(k + 1) * BS],
                        start=(k == 0), stop=(k == KT - 1))
            # stage 2: v = lrelu(z1) * s2 + o2 ; y = v.T @ W2 + b2
            vt = sb.tile([128, 3 * BS], mybir.dt.float32)
            for m in range(3):
                zs = z1ps[m][:, :]
                vs = vt[:, m * BS:(m + 1) * BS]
                nc.vector.tensor_scalar_mul(vs, zs, 0.3)
                nc.vector.tensor_tensor(vs, zs, vs, mybir.AluOpType.max)
                nc.vector.tensor_scalar(vs, vs, s2t[:, m:m + 1], o2t[:, m:m + 1],
                                        mybir.AluOpType.mult, mybir.AluOpType.add)
            yp = ps.tile([1, BS], mybir.dt.float32)
            for m in range(3):
                nc.tensor.matmul(yp[:, :], w2t[:, m:m + 1], vt[:, m * BS:(m + 1) * BS],
                                 start=(m == 0), stop=(m == 2))
            yt = sb.tile([1, BS], mybir.dt.float32)
            nc.vector.tensor_scalar_add(yt[:, :], yp[:, :], b2t[:1, :1])
            nc.sync.dma_start(out=y[:, :], in_=yt[:, :])
    nc.compile()
    return nc


def _wn(w, g):
    n = np.sqrt(np.maximum((w.astype(np.float64) ** 2).sum(axis=0, keepdims=True), 1e-12))
    return (w * (g / n)).astype(np.float32)




def _get_recurrence_fn():
    if "rec" in _BASS_CACHE:
        return _BASS_CACHE["rec"]
    import jax
    import jax.numpy as jnp
    from jax.experimental.shard_map import shard_map
    from jax.sharding import PartitionSpec as P

    devs = jax.devices()[:N_CORES]
    mesh = jax.sharding.Mesh(np.array(devs), ("d",))

    def mlstm_last(xs, idx, wxn_, whn_, wmxn_, wmhn_, b_):
        xsT = jnp.swapaxes(xs, 0, 1)

        def step(carry, x):
            c, h = carry
            m = (x @ wmxn_) * (h @ wmhn_)
            z = x @ wxn_ + m @ whn_ + b_
            i, f, o, u = jnp.split(z, 4, axis=1)
            c = jax.nn.sigmoid(f) * c + jax.nn.sigmoid(i) * jnp.tanh(u)
            h = jax.nn.sigmoid(o) * jnp.tanh(c)
            return (c, h), h

        bs = xs.shape[0]
        init = (jnp.zeros((bs, H), xs.dtype), jnp.zeros((bs, H), xs.dtype))
        _, hs = jax.lax.scan(step, init, xsT)
        hs = jnp.swapaxes(hs, 0, 1)
        return jnp.take_along_axis(hs, idx[:, None, None], axis=1)[:, 0]

    @jax.jit
    def run(epi_e, tot_e, ei_, ti_, wxn_, whn_, wmxn_, wmhn_, b_):
        f = shard_map(
            lambda ee, te, e_i, t_i: (
                mlstm_last(ee, e_i, wxn_, whn_, wmxn_, wmhn_, b_),
                mlstm_last(te, t_i, wxn_, whn_, wmxn_, wmhn_, b_),
            ),
            mesh=mesh,
            in_specs=(P("d"), P("d"), P("d"), P("d")),
            out_specs=(P("d"), P("d")),
            check_rep=False,
        )
        return f(epi_e, tot_e, ei_, ti_)

    _BASS_CACHE["rec"] = run
    return run


def kernel(epitope_x, left_antigen_x, right_antigen_x, total_antigen_x, embed,
           wx, wh, wmx, wmh, b, gx, gh, gmx, gmh,
           bn1_gamma, bn1_beta, bn1_mean, bn1_var, W1, b1,
           bn2_gamma, bn2_beta, bn2_mean, bn2_var, W2, b2):
    import jax
    import jax.numpy as jnp
    from concourse import bass_utils

    epitope_x = np.asarray(epitope_x)
    left_antigen_x = np.asarray(left_antigen_x)
    right_antigen_x = np.asarray(right_antigen_x)
    total_antigen_x = np.asarray(total_antigen_x)
    embed = np.asarray(embed, np.float32)
    wxn = _wn(np.asarray(wx, np.float32), np.asarray(gx, np.float32))
    whn = _wn(np.asarray(wh, np.float32), np.asarray(gh, np.float32))
    wmxn = _wn(np.asarray(wmx, np.float32), np.asarray(gmx, np.float32))
    wmhn = _wn(np.asarray(wmh, np.float32), np.asarray(gmh, np.float32))
    bv = np.asarray(b, np.float32)

    epi_len = (epitope_x != PAD).sum(axis=1).astype(np.int64)
    left_len = np.maximum((left_antigen_x != PAD).sum(axis=1), 1).astype(np.int64)
    right_len = np.maximum((right_antigen_x != PAD).sum(axis=1), 1).astype(np.int64)
    tot_len = epi_len + left_len + right_len
    ei = np.clip(epi_len - 1, 0, T_EPI - 1)
    ti = np.clip(tot_len - 1, 0, T_TOT - 1)

    epi_emb = embed[epitope_x]      # [B, 25, 10]
    tot_emb = embed[total_antigen_x]  # [B, 153, 10]

    # ---- recurrence: data-parallel across the 8 cores via jax shard_map ----
    run = _get_recurrence_fn()

    epi_h, tot_h = run(jnp.asarray(epi_emb), jnp.asarray(tot_emb),
                       jnp.asarray(ei.astype(np.int32)), jnp.asarray(ti.astype(np.int32)),
                       jnp.asarray(wxn), jnp.asarray(whn), jnp.asarray(wmxn),
                       jnp.asarray(wmhn), jnp.asarray(bv))
    epi_h = np.asarray(epi_h)
    tot_h = np.asarray(tot_h)

    x = np.concatenate([tot_h, epi_h], axis=-1)  # [B, 3800]

    # ---- classifier on device via Bass SPMD ----
    if "clf" not in _BASS_CACHE:
        _BASS_CACHE["clf"] = _build_classifier_kernel()
    nc = _BASS_CACHE["clf"]

    ICP = 30 * 128  # 3840
    s1v = (np.asarray(bn1_gamma, np.float32) /
           np.sqrt(np.asarray(bn1_var, np.float32) + 1e-3))
    o1v = np.asarray(bn1_beta, np.float32) - np.asarray(bn1_mean, np.float32) * s1v
    s2v = (np.asarray(bn2_gamma, np.float32) /
           np.sqrt(np.asarray(bn2_var, np.float32) + 1e-3))
    o2v = np.asarray(bn2_beta, np.float32) - np.asarray(bn2_mean, np.float32) * s2v
    # fold b1 into stage-2 input: v = lrelu(z1 + b1)*s2+o2 -> shift o? b1 enters
    # before lrelu, so fold b1 into W1 via an extra input row? Simplest: add a
    # constant feature: append row with value 1 to xT and b1 row to W1.
    s1p = np.zeros((ICP, 1), np.float32)
    o1p = np.zeros((ICP, 1), np.float32)
    s1p[:IC, 0] = s1v
    o1p[:IC, 0] = o1v
    # constant-one feature at row IC: s1=0, o1=1 -> u=1; W1 row = b1
    s1p[IC, 0] = 0.0
    o1p[IC, 0] = 1.0
    w1p = np.zeros((ICP, 384), np.float32)
    w1p[:IC, :380] = np.asarray(W1, np.float32)
    w1p[IC, :380] = np.asarray(b1, np.float32)
    s2p = np.zeros((384, 1), np.float32)
    o2p = np.zeros((384, 1), np.float32)
    s2p[:380, 0] = s2v
    o2p[:380, 0] = o2v
    w2p = np.zeros((384, 1), np.float32)
    w2p[:380, 0] = np.asarray(W2, np.float32)[:, 0]
    b2p = np.asarray(b2, np.float32).reshape(1, 1)

    BS = B // N_CORES
    in_maps = []
    for c in range(N_CORES):
        xs = np.zeros((ICP, BS), np.float32)
        xs[:IC, :] = x[c * BS:(c + 1) * BS, :].T
        xs[IC, :] = 1.0
        in_maps.append({
            "xT": xs, "w1": w1p, "s1": s1p, "o1": o1p,
            "s2": s2p, "o2": o2p, "w2": w2p, "b2": b2p,
        })
    res = bass_utils.run_bass_kernel_spmd(nc, in_maps, core_ids=list(range(N_CORES)))
    y = np.concatenate([res.results[c]["y"][0] for c in range(N_CORES)])
    return y.astype(np.float32)
